# revision 1
# baseline (speedup 1.0000x reference)
"""CropAndResize (tf.image.crop_and_resize semantics, bilinear, extrap=0)
Trainium2 Bass kernel, data-parallel over 8 NeuronCores.

Full inputs:  img (4,512,64,64) f32, rois (4,300,4) f32, input_image (4,3,1024,1024) f32
Full output:  (4,300,512,7,7) f32

Sharding: core c handles image n = c//2 and that image's roi slice
[ (c%2)*150 : (c%2)*150+150 ] (padded to 160 = 10 batches of 16).

Per-core device program (fp16 compute, f32 in/out):
  1. img NCHW f32 -> SBUF -> cast fp16 -> xbar DMA-transpose into the
     gather layout img_g[p, j, c] = row(hw=j*128+p), 1024B/row.
  2. RoI prep on DVE in [49pt, 160roi] layout: sample coords, clipped
     floors, lerp weights folded with the validity mask, gather indices.
  3. Index wrap into the dma_gather int16 [16-wrapped, replicated] layout
     and weight flatten to a per-corner row, via small DMAs.
  4. Per batch (16 rois): 4 SBUF-source transpose-mode dma_gathers
     (channels land on partitions), PE ones-matmul broadcast of weights,
     DVE lerp: out = sum_k T_k * W_k, write f32 out with strided DMA.
"""

import os
import sys

import numpy as np

_RL_REPO_CANDIDATES = ["/opt/trn_rl_repo", "/root/.axon_site/_ro/trn_rl_repo"]
for _p in _RL_REPO_CANDIDATES:
    if os.path.isdir(_p) and _p not in sys.path:
        sys.path.insert(0, _p)

import ml_dtypes  # noqa: E402

# ---------------------------------------------------------------- constants
N_CORES = 8
N, C, H, W = 4, 512, 64, 64
B = 300
POOL = 7
PTS = POOL * POOL  # 49
IH, IW = 1024.0, 1024.0
R_CORE = B // 2          # 150 real rois per core
R_PAD = 160              # padded roi count per core
RB = 8                   # rois per batch
NBATCH = R_PAD // RB     # 10
SLOT = 64                # per-roi slot stride in the gather index space
NIDX = RB * SLOT         # 1024 gather indices per batch (mult of 128)
SPB = NIDX // 16         # 64 wrapped-idx free slots per batch
HW = H * W               # 4096

_prog_cache = {}


def _build_program():
    import concourse.bass as bass
    import concourse.bacc as bacc
    import concourse.mybir as mybir
    import concourse.tile as tile

    f32 = mybir.dt.float32
    f16 = mybir.dt.float16
    i16 = mybir.dt.int16
    Alu = mybir.AluOpType

    nc = bacc.Bacc("TRN2", target_bir_lowering=False, debug=False,
                   num_devices=N_CORES)

    img_in = nc.dram_tensor("img", (C, HW), f32, kind="ExternalInput")
    rois_in = nc.dram_tensor("rois", (R_PAD, 4), f32, kind="ExternalInput")
    consts_in = nc.dram_tensor("consts", (1, 256), f32, kind="ExternalInput")
    out_t = nc.dram_tensor("out", (R_CORE, C, POOL, POOL), f32,
                           kind="ExternalOutput")

    with tile.TileContext(nc) as tc:
        _body(tc, nc, bass, mybir, tile, img_in, rois_in, consts_in, out_t,
              f32, f16, i16, Alu)

    nc.compile()
    return nc


def _body(tc, nc, bass, mybir, tile, img_in, rois_in, consts_in, out_t,
          f32, f16, i16, Alu):
    from contextlib import ExitStack
    ctx = ExitStack()
    with ctx:
        import os as _os
        GB = int(_os.environ.get("K_GBUFS", "3"))
        WB = int(_os.environ.get("K_WBUFS", "3"))
        OB = int(_os.environ.get("K_OBUFS", "2"))
        PB = int(_os.environ.get("K_PBUFS", "4"))
        const_pool = ctx.enter_context(tc.tile_pool(name="const", bufs=1))
        prep_pool = ctx.enter_context(tc.tile_pool(name="prep", bufs=1))
        imgstage = ctx.enter_context(tc.tile_pool(name="imgstage", bufs=2))
        gather_pool = ctx.enter_context(tc.tile_pool(name="gather", bufs=GB))
        w_pool = ctx.enter_context(tc.tile_pool(name="wts", bufs=WB))
        wrow_pool = ctx.enter_context(tc.tile_pool(name="wrow", bufs=WB))
        o_pool = ctx.enter_context(tc.tile_pool(name="outs", bufs=OB))
        dram_pool = ctx.enter_context(
            tc.tile_pool(name="dram", bufs=1, space="DRAM"))
        psum_pool = ctx.enter_context(
            tc.tile_pool(name="psum", bufs=PB, space="PSUM"))

        # ------------------------------------------------ constants
        # consts row: [0:49]=g_y per pt, [49:98]=g_x per pt, [98:226]=ones
        gy_col = const_pool.tile([PTS, 1], f32, tag="gy")
        gx_col = const_pool.tile([PTS, 1], f32, tag="gx")
        # strided loads: partition p <- consts[0, p] / consts[0, 49+p]
        nc.sync.dma_start(gy_col[:, :], consts_in.ap()[0:1, 0:PTS].rearrange(
            "a p -> p a"))
        nc.sync.dma_start(gx_col[:, :], consts_in.ap()[0:1, PTS:2 * PTS]
                          .rearrange("a p -> p a"))
        ones_f32 = const_pool.tile([1, 128], f32, tag="ones32")
        nc.sync.dma_start(ones_f32[:, :], consts_in.ap()[0:1, 98:226])
        ones16 = const_pool.tile([1, 128], f16, tag="ones16")
        nc.vector.tensor_copy(ones16[:, :], ones_f32[:, :])

        # ------------------------------------------------ image prep
        # img_nhwc[hw, c] fp16 in DRAM; built via SBUF xbar transpose:
        # img_g[p, j, c] = img row hw=j*128+p -> DRAM row-major write
        img_nhwc = dram_pool.tile([HW, C], f16, name="img_nhwc")
        img_g = const_pool.tile([128, 32, C], f16, tag="imgg")
        for cs in range(4):
            stage32 = imgstage.tile([128, HW], f32, tag="s32")
            nc.sync.dma_start(stage32[:, :],
                              img_in.ap()[cs * 128:(cs + 1) * 128, :])
            stage16 = imgstage.tile([128, HW], f16, tag="s16")
            nc.vector.tensor_copy(stage16[:, :], stage32[:, :])
            nc.sync.dma_start_transpose(
                img_g[:, :, cs * 128:(cs + 1) * 128], stage16[:, :])
        nc.sync.dma_start(
            img_nhwc[:, :].rearrange("(j p) c -> p j c", p=128),
            img_g[:, :, :])

        # ------------------------------------------------ roi prep
        # roisT[0, c*R_PAD + r] = rois[r, c]
        roisT = prep_pool.tile([1, 4 * R_PAD], f32, tag="roisT")
        nc.sync.dma_start(roisT[:, :].rearrange("o (c r) -> o c r", c=4),
                          rois_in.ap()[:, :].rearrange("r c -> c r"))

        # bc row: [y1n, x1n, dyn, dxn] each R_PAD wide
        bc = prep_pool.tile([64, 4 * R_PAD], f32, tag="bc")
        inv_h = 1.0 / (IH - 1.0)
        inv_w = 1.0 / (IW - 1.0)
        r0 = bc[0:1, :]
        nc.vector.tensor_scalar_mul(r0[:, 0:R_PAD], roisT[:, 0:R_PAD], inv_h)
        nc.vector.tensor_scalar_mul(r0[:, R_PAD:2 * R_PAD],
                                    roisT[:, R_PAD:2 * R_PAD], inv_w)
        tmp = prep_pool.tile([1, 2 * R_PAD], f32, tag="tmp2")
        nc.vector.tensor_scalar_mul(tmp[:, 0:R_PAD],
                                    roisT[:, 2 * R_PAD:3 * R_PAD], inv_h)
        nc.vector.tensor_scalar_mul(tmp[:, R_PAD:2 * R_PAD],
                                    roisT[:, 3 * R_PAD:4 * R_PAD], inv_w)
        nc.vector.tensor_sub(r0[:, 2 * R_PAD:3 * R_PAD], tmp[:, 0:R_PAD],
                             r0[:, 0:R_PAD])
        nc.vector.tensor_sub(r0[:, 3 * R_PAD:4 * R_PAD],
                             tmp[:, R_PAD:2 * R_PAD], r0[:, R_PAD:2 * R_PAD])
        # broadcast row 0 -> 64 partitions (need 49) by doubling
        for k in range(6):
            lo, hi = 1 << k, min(2 << k, 64)
            nc.sync.dma_start(bc[lo:hi, :], bc[0:lo, :][0:hi - lo, :])

        y1n = bc[0:PTS, 0:R_PAD]
        x1n = bc[0:PTS, R_PAD:2 * R_PAD]
        dyn = bc[0:PTS, 2 * R_PAD:3 * R_PAD]
        dxn = bc[0:PTS, 3 * R_PAD:4 * R_PAD]

        def ptile(nm, dt_=None):
            return prep_pool.tile([PTS, R_PAD], dt_ or f32, tag=nm, name=nm)

        def prep_axis(ax, gcol, lo_n, d_n, hdim):
            """returns (c0f, cbf, lc, mc) tiles [49, R_PAD] f32"""
            inn = ptile(f"inn{ax}")
            nc.vector.scalar_tensor_tensor(inn[:, :], d_n, gcol[:, :], lo_n,
                                           Alu.mult, Alu.add)
            nc.vector.tensor_scalar_mul(inn[:, :], inn[:, :], hdim - 1.0)
            cc = ptile(f"cc{ax}")
            nc.vector.tensor_scalar(cc[:, :], inn[:, :], 0.0, hdim - 1.0,
                                    Alu.max, Alu.min)
            # exact floor for 0<=x<2^22: t=(x+2^23)-2^23 is round-nearest;
            # subtract 1 where t > x
            rnd = ptile(f"rnd{ax}")
            nc.vector.tensor_scalar(rnd[:, :], cc[:, :], 8388608.0, 8388608.0,
                                    Alu.add, Alu.subtract)
            gt = ptile(f"gt{ax}")
            nc.vector.tensor_tensor(gt[:, :], rnd[:, :], cc[:, :], Alu.is_gt)
            c0f = ptile(f"c0f{ax}")
            nc.vector.tensor_sub(c0f[:, :], rnd[:, :], gt[:, :])
            cbf = ptile(f"cbf{ax}")
            nc.vector.tensor_scalar(cbf[:, :], c0f[:, :], 1.0, hdim - 1.0,
                                    Alu.add, Alu.min)
            lc = ptile(f"lc{ax}")
            nc.vector.tensor_sub(lc[:, :], inn[:, :], c0f[:, :])
            m1 = ptile(f"m1{ax}")
            nc.vector.tensor_scalar(m1[:, :], inn[:, :], 0.0, None, Alu.is_ge)
            m2 = ptile(f"m2{ax}")
            nc.vector.tensor_scalar(m2[:, :], inn[:, :], hdim - 1.0, None,
                                    Alu.is_le)
            mc = ptile(f"mc{ax}")
            nc.vector.tensor_mul(mc[:, :], m1[:, :], m2[:, :])
            return inn, c0f, cbf, lc, mc

        _, y0f, ybf, ly, my = prep_axis("y", gy_col, y1n, dyn, float(H))
        _, x0f, xbf, lx, mx = prep_axis("x", gx_col, x1n, dxn, float(W))

        def ab(ax_, lc, mc):
            a = ptile(f"a{ax_}")
            nc.vector.tensor_scalar(a[:, :], lc[:, :], -1.0, 1.0, Alu.mult,
                                    Alu.add)
            nc.vector.tensor_mul(a[:, :], a[:, :], mc[:, :])
            b = ptile(f"b{ax_}")
            nc.vector.tensor_mul(b[:, :], lc[:, :], mc[:, :])
            return a, b

        ay, by = ab("y", ly, my)
        ax, bx = ab("x", lx, mx)

        # per-corner weights (fp16) and indices (int16)
        corners = []  # (w16 tile, idx16 tile)
        for kc, (wy, wx_, yf, xf) in enumerate(
                ((ay, ax, y0f, x0f), (ay, bx, y0f, xbf),
                 (by, ax, ybf, x0f), (by, bx, ybf, xbf))):
            w16 = ptile(f"w16_{kc}", f16)
            nc.vector.tensor_mul(w16[:, :], wy[:, :], wx_[:, :])
            idxf = ptile(f"idxf{kc}")
            nc.vector.scalar_tensor_tensor(idxf[:, :], yf[:, :], float(W),
                                           xf[:, :], Alu.mult, Alu.add)
            idx16 = ptile(f"idx16_{kc}", i16)
            nc.vector.tensor_copy(idx16[:, :], idxf[:, :])
            corners.append((w16, idx16))

        # ------------------------------------------------ idx wrap + W flatten
        # gather order within batch b: j = rl*64 + pt  (rl<16, pt<49 valid)
        # wrapped: partition p = pt%16 (q=pt//16<4), slot s = rl*4 + q
        # idxw[k] free layout: [b(10), s(64)]
        idxw = const_pool.tile([128, 4, NBATCH, SPB], i16, tag="idxw")
        nc.gpsimd.memset(idxw[:, :, :, :], 0)
        # wflat: partition k holds corner k's flat row [b(10), rl(16), pt-slot(64)]
        wdram = dram_pool.tile([4, NBATCH * NIDX], f16, name="wdram")
        for k, (w16, idx16) in enumerate(corners):
            # idx wrap: dst[p, k, b, rl*4+q] = idx16[q*16+p, b*16+rl]
            for q in range(4):
                npq = min(16, PTS - q * 16)  # 16,16,16,1
                src = idx16[q * 16:q * 16 + npq, :].rearrange(
                    "p (b r) -> p b r", b=NBATCH)
                dst = idxw[0:npq, k, :, :].rearrange(
                    "p b (r q) -> p b r q", q=4)[:, :, :, q]
                nc.sync.dma_start(dst, src)
            # w flatten: wflat[k, b*1024 + rl*64 + pt] = w16[pt, b*16+rl]
            # dst iterated (s, b, r) to match src element order (p, b, r)
            dstw = wdram[k:k + 1, :].rearrange(
                "o (b r s) -> o s b r", b=NBATCH, r=RB)[:, 0:PTS, :, :]
            nc.sync.dma_start(dstw, w16[:, :].rearrange(
                "p (b r) -> p b r", b=NBATCH))
        for k in range(3):
            lo, hi = 16 << k, 32 << k
            nc.sync.dma_start(idxw[lo:hi, :, :, :], idxw[0:hi - lo, :, :, :])

        # ------------------------------------------------ main loop
        for b in range(NBATCH):
            # rois beyond R_CORE are host-side padding; skip fully-pad batches
            nv = RB if (b + 1) * RB <= R_CORE else R_CORE - b * RB
            if nv <= 0:
                continue
            ob = o_pool.tile([128, 4, RB, PTS], f16, tag="O")
            for k in range(4):
                tk = gather_pool.tile([128, 4, NIDX], f16, tag="T")
                nc.gpsimd.dma_gather(
                    tk[:, :, :], img_nhwc[:, :], idxw[:, k, b, :],
                    NIDX, NIDX, C,
                    transpose=True,
                )
                wrow = wrow_pool.tile([1, NIDX], f16, tag="wr")
                nc.sync.dma_start(wrow[:, :],
                                  wdram[k:k + 1, b * NIDX:(b + 1) * NIDX])
                wk = w_pool.tile([128, NIDX], f16, tag="W")
                ps = psum_pool.tile([128, NIDX], f32, tag="ps")
                nc.tensor.matmul(ps[:, :], ones16[:, :], wrow[:, :],
                                 start=True, stop=True)
                nc.scalar.copy(wk[:, :], ps[:, :])
                # valid-slot views [128, 4, RB, PTS]
                tv = tk[:, :, :].rearrange("p e (r s) -> p e r s",
                                           r=RB)[:, :, :, 0:PTS]
                wv = wk[:, :].rearrange("p (r s) -> p r s",
                                        r=RB)[:, :, 0:PTS]
                wv4 = wv  # broadcast over e by explicit per-e ops
                if k == 0:
                    for e in range(4):
                        nc.vector.tensor_mul(ob[:, e, :, :], tv[:, e, :, :],
                                             wv4)
                else:
                    for e in range(4):
                        nc.vector.tensor_mul(tv[:, e, :, :], tv[:, e, :, :],
                                             wv4)
                    nc.vector.tensor_add(ob[:, :, :, :], ob[:, :, :, :], tv)

            # output write with cast fp16 -> f32
            # dst out[b*16+rl, e*128+p, py, px]; 3-dim AP limit -> per-e DMA
            dste = out_t.ap()[b * RB:b * RB + nv, :, :, :].rearrange(
                "r (e p) py px -> p e r (py px)", e=4)
            for e in range(4):
                nc.gpsimd.dma_start(dste[:, e, :, :], ob[:, e, 0:nv, :])


def _get_program():
    if "nc" not in _prog_cache:
        _prog_cache["nc"] = _build_program()
    return _prog_cache["nc"]


def _make_consts():
    consts = np.zeros((1, 256), dtype=np.float32)
    g = (np.arange(POOL, dtype=np.float32) / np.float32(POOL - 1.0)).astype(
        np.float32)
    gy = np.repeat(g, POOL)   # g[pt//7]
    gx = np.tile(g, POOL)     # g[pt%7]
    consts[0, 0:PTS] = gy
    consts[0, PTS:2 * PTS] = gx
    consts[0, 98:226] = 1.0
    return consts


def kernel(img: np.ndarray, rois: np.ndarray,
           input_image: np.ndarray) -> np.ndarray:
    from concourse.bass_utils import run_bass_kernel_spmd

    nc = _get_program()
    consts = _make_consts()
    in_maps = []
    for c in range(N_CORES):
        n, half = c // 2, c % 2
        rpad = np.zeros((R_PAD, 4), dtype=np.float32)
        rpad[:R_CORE] = rois[n, half * R_CORE:(half + 1) * R_CORE]
        in_maps.append({
            "img": np.ascontiguousarray(
                img[n].reshape(C, HW).astype(np.float32)),
            "rois": rpad,
            "consts": consts,
        })
    res = run_bass_kernel_spmd(nc, in_maps, core_ids=list(range(N_CORES)))
    out = np.empty((N, B, C, POOL, POOL), dtype=np.float32)
    for c in range(N_CORES):
        n, half = c // 2, c % 2
        out[n, half * R_CORE:(half + 1) * R_CORE] = res.results[c]["out"]
    return out



# revision 5
# speedup vs baseline: 2.7041x; 2.7041x over previous
"""CropAndResize (tf.image.crop_and_resize semantics, bilinear, extrap=0)
Trainium2 Bass kernel, data-parallel over 8 NeuronCores.

Full inputs:  img (4,512,64,64) f32, rois (4,300,4) f32, input_image (4,3,1024,1024) f32
Full output:  (4,300,512,7,7) f32

Sharding: core c handles image n = c//2, roi slice [(c%2)*150 : +150].

Host prep (numpy, per core):
  - imgt[hw, q] fp16 token table: payload position q = e*128+p holds channel
    4p+e (so after the transpose-gather, SBUF partition p carries the four
    adjacent channels 4p..4p+3 -> 784B-contiguous output descriptors).
  - Sample coords/weights mirror the reference math in f32; the validity
    mask and lerp factors fold into one fp16 weight per (corner, point).
  - Gather indices in the dma_gather wrapped-int16 layout, weights as a
    flat f16 row. 10 out-batches of 15 rois (735 points padded to 736);
    each out-batch gathers in 6 chunks (5x128 + 1x96 points, corner-major
    within a chunk) to stay under the 512-descriptor SWDGE ring limit.

Device program (per core, per out-batch):
  1. 6 dma_gathers (transpose mode) straight from DRAM imgt ->
     tk[p, e, (k, i)] fp16 per chunk.
  2. Per chunk: PE ones-matmul broadcasts the weight row (PSUM),
     Activation copies PSUM -> fp16 wk; DVE multiplies the whole chunk by
     its weights in one op, writing into prod[p, e, k, i] (corner-planar).
  3. DVE sums the 4 corner planes (3 adds, fp16 2x).
  4. Activation casts/permutes acc[p, e, (r,pt)] -> ob2[p, r, (e,pt)] f32.
  5. sync DMA ob2 -> out[r0:r0+15] with 784B contiguous descriptors.
"""

import os
import sys

import numpy as np

_RL_REPO_CANDIDATES = ["/opt/trn_rl_repo", "/root/.axon_site/_ro/trn_rl_repo"]
for _p in _RL_REPO_CANDIDATES:
    if os.path.isdir(_p) and _p not in sys.path:
        sys.path.insert(0, _p)

# ---------------------------------------------------------------- constants
N_CORES = 8
N, C, H, W = 4, 512, 64, 64
B = 300
POOL = 7
PTS = POOL * POOL          # 49
IH, IW = 1024.0, 1024.0
R_CORE = B // 2            # 150 rois per core
HW = H * W                 # 4096

RB = 15                    # rois per out-batch
NBATCH = R_CORE // RB      # 10
NP = RB * PTS              # 735 points per out-batch
NP_PAD = 736               # padded (mult of 32)
CHUNKS = (128, 128, 128, 128, 128, 96)   # points per gather chunk
assert sum(CHUNKS) == NP_PAD
NIDX_B = 4 * NP_PAD        # 2944 gather rows per out-batch
SPB = NIDX_B // 16         # 184 wrapped slots per out-batch
S_TOT = NBATCH * SPB       # 1840
J_TOT = NBATCH * NIDX_B    # 29440

_prog_cache = {}


def _build_program():
    import concourse.bass as bass
    import concourse.bacc as bacc
    import concourse.mybir as mybir
    import concourse.tile as tile

    f32 = mybir.dt.float32
    f16 = mybir.dt.float16
    i16 = mybir.dt.int16
    Alu = mybir.AluOpType

    nc = bacc.Bacc("TRN2", target_bir_lowering=False, debug=False,
                   num_devices=N_CORES)

    imgt = nc.dram_tensor("imgt", (HW, C), f16, kind="ExternalInput")
    idxt = nc.dram_tensor("idxt", (128, S_TOT), i16, kind="ExternalInput")
    wrow_d = nc.dram_tensor("wrow", (1, J_TOT), f16, kind="ExternalInput")
    ones_d = nc.dram_tensor("ones", (1, 128), f16, kind="ExternalInput")
    out_t = nc.dram_tensor("out", (R_CORE, C, PTS), f32,
                           kind="ExternalOutput")

    with tile.TileContext(nc) as tc:
        _body(tc, nc, bass, mybir, tile, imgt, idxt, wrow_d, ones_d, out_t,
              f32, f16, i16, Alu)

    nc.compile()
    return nc


def _body(tc, nc, bass, mybir, tile, imgt, idxt, wrow_d, ones_d, out_t,
          f32, f16, i16, Alu):
    from contextlib import ExitStack
    ctx = ExitStack()
    with ctx:
        const_pool = ctx.enter_context(tc.tile_pool(name="const", bufs=1))
        gather_pool = ctx.enter_context(tc.tile_pool(name="gather", bufs=3))
        wrow_pool = ctx.enter_context(tc.tile_pool(name="wrow", bufs=2))
        wk_pool = ctx.enter_context(tc.tile_pool(name="wk", bufs=3))
        prod_pool = ctx.enter_context(tc.tile_pool(name="prod", bufs=2))
        acc_pool = ctx.enter_context(tc.tile_pool(name="acc", bufs=2))
        ob_pool = ctx.enter_context(tc.tile_pool(name="ob", bufs=2))
        psum_pool = ctx.enter_context(
            tc.tile_pool(name="psum", bufs=4, space="PSUM"))

        # ---- constants
        idx_s = const_pool.tile([128, S_TOT], i16, tag="idx")
        nc.sync.dma_start(idx_s[:, :], idxt.ap()[:, :])
        ones_s = const_pool.tile([1, 128], f16, tag="ones")
        nc.sync.dma_start(ones_s[:, :], ones_d.ap()[:, :])

        for b in range(NBATCH):
            r0 = b * RB

            # prod[p, e, k, i]: corner-planar weighted gather products
            prod = prod_pool.tile([128, 4, 4, NP_PAD], f16, tag="P")

            # weight row for the whole out-batch
            wr = wrow_pool.tile([1, NIDX_B], f16, tag="wr")
            nc.sync.dma_start(wr[:, :],
                              wrow_d.ap()[0:1, b * NIDX_B:(b + 1) * NIDX_B])

            off = 0  # point offset within the out-batch
            for g, ng in enumerate(CHUNKS):
                nidx = 4 * ng
                s0 = b * SPB + off * 4 // 16
                j0 = 4 * off
                tkg = gather_pool.tile([128, 4, nidx], f16, tag=f"T{ng}")
                nc.gpsimd.dma_gather(
                    tkg[:, :, :], imgt.ap()[:, :],
                    idx_s[:, s0:s0 + nidx // 16],
                    nidx, nidx, C, transpose=True,
                )
                ps = psum_pool.tile([128, nidx], f32, tag=f"ps{ng}")
                nc.tensor.matmul(ps[:, :], ones_s[:, :],
                                 wr[:, j0:j0 + nidx], start=True, stop=True)
                wk = wk_pool.tile([128, nidx], f16, tag=f"wk{ng}")
                nc.scalar.copy(wk[:, :], ps[:, :])
                wkb = wk[:, :].unsqueeze(1).broadcast_to([128, 4, nidx])
                # one mul per chunk; dst view splits (k,i) into planes
                src = tkg[:, :, :].rearrange("p e (k i) -> p e k i", k=4)
                dst = prod[:, :, :, off:off + ng]
                nc.vector.tensor_tensor(
                    dst, src, wkb.rearrange("p e (k i) -> p e k i", k=4),
                    Alu.mult)
                off += ng

            # corner reduction: acc = ((P0+P1)+P2)+P3
            acc = acc_pool.tile([128, 4, NP_PAD], f16, tag="A")
            nc.vector.tensor_tensor(acc[:, :, :], prod[:, :, 0, :],
                                    prod[:, :, 1, :], Alu.add)
            nc.vector.tensor_tensor(acc[:, :, :], acc[:, :, :],
                                    prod[:, :, 2, :], Alu.add)
            nc.vector.tensor_tensor(acc[:, :, :], acc[:, :, :],
                                    prod[:, :, 3, :], Alu.add)

            # cast + permute (e, r, pt) -> (r, e, pt) on Activation
            ob2 = ob_pool.tile([128, RB, 4 * PTS], f32, tag="O")
            srcv = acc[:, :, 0:NP].rearrange("p e (r q) -> p e r q", r=RB)
            dstv = ob2[:, :, :].rearrange("p r (e q) -> p e r q", e=4)
            nc.scalar.copy(dstv, srcv)

            # output write: contiguous 784B per (partition, roi)
            dram = out_t.ap()[r0:r0 + RB, :, :].rearrange(
                "r (p e) q -> p r (e q)", e=4)
            nc.sync.dma_start(dram, ob2[:, :, :])


def _get_program():
    if "nc" not in _prog_cache:
        _prog_cache["nc"] = _build_program()
    return _prog_cache["nc"]


# Channel permutation: payload position q = e*128+p holds channel 4p+e.
_POS = np.arange(C)
_CHAN_OF_POS = 4 * (_POS % 128) + _POS // 128  # [512] channel at position q


def _host_tables(rois_n: np.ndarray):
    """Mirror the reference coordinate math in f32; return wrapped int16
    gather indices [128, S_TOT] and folded fp16 corner weights [1, J_TOT].

    Flat j order: batch-major, then chunk, then corner-major within chunk:
    j = b*NIDX_B + 4*off(g) + k*ng + ii.
    """
    r = rois_n.astype(np.float32)
    g = np.arange(POOL, dtype=np.float32) / np.float32(POOL - 1.0)
    y1 = r[:, 0] / np.float32(IH - 1.0)
    x1 = r[:, 1] / np.float32(IW - 1.0)
    y2 = r[:, 2] / np.float32(IH - 1.0)
    x2 = r[:, 3] / np.float32(IW - 1.0)
    in_y = (y1[:, None] + (y2 - y1)[:, None] * g[None, :]) * np.float32(H - 1.0)
    in_x = (x1[:, None] + (x2 - x1)[:, None] * g[None, :]) * np.float32(W - 1.0)
    val_y = (in_y >= 0.0) & (in_y <= np.float32(H - 1.0))
    val_x = (in_x >= 0.0) & (in_x <= np.float32(W - 1.0))
    y0f = np.floor(in_y)
    x0f = np.floor(in_x)
    y0 = np.clip(y0f, 0, H - 1).astype(np.int64)
    x0 = np.clip(x0f, 0, W - 1).astype(np.int64)
    yb = np.minimum(y0 + 1, H - 1)
    xb = np.minimum(x0 + 1, W - 1)
    ly = (in_y - y0f).astype(np.float32)
    lx = (in_x - x0f).astype(np.float32)
    ay = (1.0 - ly) * val_y
    by = ly * val_y
    ax = (1.0 - lx) * val_x
    bx = lx * val_x

    iy0 = (y0 * W)[:, :, None]
    iyb = (yb * W)[:, :, None]
    jx0 = x0[:, None, :]
    jxb = xb[:, None, :]
    # [4, R, 49] corner indices / weights in (r, pt) point order
    idx4 = np.stack([iy0 + jx0, iy0 + jxb, iyb + jx0, iyb + jxb]).reshape(
        4, R_CORE, PTS)
    w4 = np.stack([ay[:, :, None] * ax[:, None, :],
                   ay[:, :, None] * bx[:, None, :],
                   by[:, :, None] * ax[:, None, :],
                   by[:, :, None] * bx[:, None, :]]).reshape(4, R_CORE, PTS)

    idx_flat = np.zeros(J_TOT, dtype=np.int16)
    w_flat = np.zeros(J_TOT, dtype=np.float16)
    for b in range(NBATCH):
        # flat per-batch point arrays [4, NP]
        ib = idx4[:, b * RB:(b + 1) * RB].reshape(4, NP)
        wb = w4[:, b * RB:(b + 1) * RB].reshape(4, NP)
        j0 = b * NIDX_B
        off = 0
        for ng in CHUNKS:
            nreal = min(ng, NP - off) if off < NP else 0
            for k in range(4):
                o = j0 + 4 * off + k * ng
                if nreal > 0:
                    idx_flat[o:o + nreal] = ib[k, off:off + nreal]
                    w_flat[o:o + nreal] = wb[k, off:off + nreal]
            off += ng

    # wrapped layout per batch: within batch, idx j at partition j%16,
    # slot j//16 (matches per-chunk gather slices since chunk NIDX % 16 == 0)
    idxw = np.empty((128, S_TOT), dtype=np.int16)
    for b in range(NBATCH):
        blk = idx_flat[b * NIDX_B:(b + 1) * NIDX_B].reshape(SPB, 16).T
        idxw[:, b * SPB:(b + 1) * SPB] = np.tile(blk, (8, 1))
    return idxw, w_flat.reshape(1, J_TOT)


def kernel(img: np.ndarray, rois: np.ndarray,
           input_image: np.ndarray) -> np.ndarray:
    from concourse.bass_utils import run_bass_kernel_spmd

    nc = _get_program()
    ones = np.ones((1, 128), dtype=np.float16)
    in_maps = []
    for c in range(N_CORES):
        n, half = c // 2, c % 2
        imgt = np.ascontiguousarray(
            img[n].reshape(C, HW)[_CHAN_OF_POS, :].T).astype(np.float16)
        idxw, wrow = _host_tables(
            rois[n, half * R_CORE:(half + 1) * R_CORE])
        in_maps.append({
            "imgt": imgt,
            "idxt": idxw,
            "wrow": wrow,
            "ones": ones,
        })
    res = run_bass_kernel_spmd(nc, in_maps, core_ids=list(range(N_CORES)))
    out = np.empty((N, B, C, POOL, POOL), dtype=np.float32)
    for c in range(N_CORES):
        n, half = c // 2, c % 2
        out[n, half * R_CORE:(half + 1) * R_CORE] = \
            res.results[c]["out"].reshape(R_CORE, C, POOL, POOL)
    return out


# revision 6
# speedup vs baseline: 3.0661x; 1.1339x over previous
"""CropAndResize (tf.image.crop_and_resize semantics, bilinear, extrap=0)
Trainium2 Bass kernel, data-parallel over 8 NeuronCores.

Full inputs:  img (4,512,64,64) f32, rois (4,300,4) f32, input_image (4,3,1024,1024) f32
Full output:  (4,300,512,7,7) f32

Sharding: core c handles image n = c//2, roi slice [(c%2)*150 : +150].

Host prep (numpy, per core):
  - imgt[hw, q] fp16 token table: payload position q = e*128+p holds channel
    4p+e (so after the transpose-gather, SBUF partition p carries the four
    adjacent channels 4p..4p+3 -> 784B-contiguous output descriptors).
  - Sample coords/weights mirror the reference math in f32; the validity
    mask and lerp factors fold into one fp16 weight per (corner, point).
  - Gather indices in the dma_gather wrapped-int16 layout, weights as a
    flat f16 row. 10 out-batches of 15 rois (735 points padded to 736);
    each out-batch gathers in 6 chunks (5x128 + 1x96 points, corner-major
    within a chunk) to stay under the 512-descriptor SWDGE ring limit.

Device program (per core, per out-batch):
  1. 6 dma_gathers (transpose mode) straight from DRAM imgt ->
     tk[p, e, (k, i)] fp16 per chunk.
  2. Per chunk: PE ones-matmul broadcasts the weight row (PSUM),
     Activation copies PSUM -> fp16 wk; DVE multiplies the whole chunk by
     its weights in one op, writing into prod[p, e, k, i] (corner-planar).
  3. DVE sums the 4 corner planes (3 adds, fp16 2x).
  4. Activation casts/permutes acc[p, e, (r,pt)] -> ob2[p, r, (e,pt)] f32.
  5. sync DMA ob2 -> out[r0:r0+15] with 784B contiguous descriptors.
"""

import os
import sys

import numpy as np

_RL_REPO_CANDIDATES = ["/opt/trn_rl_repo", "/root/.axon_site/_ro/trn_rl_repo"]
for _p in _RL_REPO_CANDIDATES:
    if os.path.isdir(_p) and _p not in sys.path:
        sys.path.insert(0, _p)

# ---------------------------------------------------------------- constants
N_CORES = 8
N, C, H, W = 4, 512, 64, 64
B = 300
POOL = 7
PTS = POOL * POOL          # 49
IH, IW = 1024.0, 1024.0
R_CORE = B // 2            # 150 rois per core
HW = H * W                 # 4096

RB = 15                    # rois per out-batch
NBATCH = R_CORE // RB      # 10
NP = RB * PTS              # 735 points per out-batch
NP_PAD = 736               # padded (mult of 32)
CHUNKS = (128, 128, 128, 128, 128, 96)   # points per gather chunk
assert sum(CHUNKS) == NP_PAD
NIDX_B = 4 * NP_PAD        # 2944 gather rows per out-batch
SPB = NIDX_B // 16         # 184 wrapped slots per out-batch
S_TOT = NBATCH * SPB       # 1840
J_TOT = NBATCH * NIDX_B    # 29440

_prog_cache = {}


def _build_program():
    import concourse.bass as bass
    import concourse.bacc as bacc
    import concourse.mybir as mybir
    import concourse.tile as tile

    f32 = mybir.dt.float32
    f16 = mybir.dt.float16
    i16 = mybir.dt.int16
    Alu = mybir.AluOpType

    nc = bacc.Bacc("TRN2", target_bir_lowering=False, debug=False,
                   num_devices=N_CORES)

    imgt = nc.dram_tensor("imgt", (HW, C), f16, kind="ExternalInput")
    idxt = nc.dram_tensor("idxt", (128, S_TOT), i16, kind="ExternalInput")
    wrow_d = nc.dram_tensor("wrow", (1, J_TOT), f16, kind="ExternalInput")
    ones_d = nc.dram_tensor("ones", (1, 128), f16, kind="ExternalInput")
    out_t = nc.dram_tensor("out", (R_CORE, C, PTS), f32,
                           kind="ExternalOutput")

    with tile.TileContext(nc) as tc:
        _body(tc, nc, bass, mybir, tile, imgt, idxt, wrow_d, ones_d, out_t,
              f32, f16, i16, Alu)

    nc.compile()
    return nc


def _body(tc, nc, bass, mybir, tile, imgt, idxt, wrow_d, ones_d, out_t,
          f32, f16, i16, Alu):
    from contextlib import ExitStack
    ctx = ExitStack()
    with ctx:
        const_pool = ctx.enter_context(tc.tile_pool(name="const", bufs=1))
        gather_pool = ctx.enter_context(tc.tile_pool(name="gather", bufs=2))
        wk_pool = ctx.enter_context(tc.tile_pool(name="wk", bufs=2))
        prod_pool = ctx.enter_context(tc.tile_pool(name="prod", bufs=2))
        acc_pool = ctx.enter_context(tc.tile_pool(name="acc", bufs=2))
        ob_pool = ctx.enter_context(tc.tile_pool(name="ob", bufs=2))
        psum_pool = ctx.enter_context(
            tc.tile_pool(name="psum", bufs=1, space="PSUM"))

        # ---- constants; idx split so batch 0 can gather immediately
        idx_s = const_pool.tile([128, S_TOT], i16, tag="idx")
        nc.sync.dma_start(idx_s[:, 0:SPB], idxt.ap()[:, 0:SPB])
        nc.sync.dma_start(idx_s[:, SPB:], idxt.ap()[:, SPB:])
        ones_s = const_pool.tile([1, 128], f16, tag="ones")
        nc.sync.dma_start(ones_s[:, :], ones_d.ap()[:, :])
        # all corner weights resident (one small DMA; keeps SP free of the
        # per-batch load that would queue behind output DMAs)
        wr_all = const_pool.tile([1, J_TOT], f16, tag="wr")
        nc.sync.dma_start(wr_all[:, :], wrow_d.ap()[:, :])

        for b in range(NBATCH):
            r0 = b * RB

            # prod[p, e, k, i]: corner-planar weighted gather products
            prod = prod_pool.tile([128, 4, 4, NP_PAD], f16, tag="P")

            off = 0  # point offset within the out-batch
            for g, ng in enumerate(CHUNKS):
                nidx = 4 * ng
                s0 = b * SPB + off * 4 // 16
                j0 = b * NIDX_B + 4 * off
                tkg = gather_pool.tile([128, 4, nidx], f16, tag=f"T{ng}",
                                       bufs=6 if ng == 128 else 2)
                nc.gpsimd.dma_gather(
                    tkg[:, :, :], imgt.ap()[:, :],
                    idx_s[:, s0:s0 + nidx // 16],
                    nidx, nidx, C, transpose=True,
                )
                ps = psum_pool.tile([128, nidx], f32, tag=f"ps{ng}",
                                    bufs=5 if ng == 128 else 2)
                nc.tensor.matmul(ps[:, :], ones_s[:, :],
                                 wr_all[:, j0:j0 + nidx],
                                 start=True, stop=True)
                wk = wk_pool.tile([128, nidx], f16, tag=f"wk{ng}",
                                  bufs=6 if ng == 128 else 2)
                nc.scalar.copy(wk[:, :], ps[:, :])
                wkb = wk[:, :].unsqueeze(1).broadcast_to([128, 4, nidx])
                # one mul per chunk; dst view splits (k,i) into planes
                src = tkg[:, :, :].rearrange("p e (k i) -> p e k i", k=4)
                dst = prod[:, :, :, off:off + ng]
                nc.vector.tensor_tensor(
                    dst, src, wkb.rearrange("p e (k i) -> p e k i", k=4),
                    Alu.mult)
                off += ng

            acc = acc_pool.tile([128, 4, NP_PAD], f16, tag="A")
            ob2 = ob_pool.tile([128, RB, 4 * PTS], f32, tag="O")
            # last batch: split the drain chain in two so the final
            # gather's dependents are short
            if b == NBATCH - 1:
                halves = ((0, 384, 0, 7), (384, NP_PAD, 7, RB))
            else:
                halves = ((0, NP_PAD, 0, RB),)
            for (i0, i1, ra, rb_) in halves:
                # corner reduction: acc = ((P0+P1)+P2)+P3
                nc.vector.tensor_tensor(
                    acc[:, :, i0:i1], prod[:, :, 0, i0:i1],
                    prod[:, :, 1, i0:i1], Alu.add)
                nc.vector.tensor_tensor(
                    acc[:, :, i0:i1], acc[:, :, i0:i1],
                    prod[:, :, 2, i0:i1], Alu.add)
                nc.vector.tensor_tensor(
                    acc[:, :, i0:i1], acc[:, :, i0:i1],
                    prod[:, :, 3, i0:i1], Alu.add)
                # cast + permute (e, r, pt) -> (r, e, pt) on Activation
                srcv = acc[:, :, ra * PTS:rb_ * PTS].rearrange(
                    "p e (r q) -> p e r q", r=rb_ - ra)
                dstv = ob2[:, ra:rb_, :].rearrange(
                    "p r (e q) -> p e r q", e=4)
                nc.scalar.copy(dstv, srcv)
                # output write: contiguous 784B per (partition, roi)
                dram = out_t.ap()[r0 + ra:r0 + rb_, :, :].rearrange(
                    "r (p e) q -> p r (e q)", e=4)
                nc.sync.dma_start(dram, ob2[:, ra:rb_, :])


def _get_program():
    if "nc" not in _prog_cache:
        _prog_cache["nc"] = _build_program()
    return _prog_cache["nc"]


# Channel permutation: payload position q = e*128+p holds channel 4p+e.
_POS = np.arange(C)
_CHAN_OF_POS = 4 * (_POS % 128) + _POS // 128  # [512] channel at position q


def _host_tables(rois_n: np.ndarray):
    """Mirror the reference coordinate math in f32; return wrapped int16
    gather indices [128, S_TOT] and folded fp16 corner weights [1, J_TOT].

    Flat j order: batch-major, then chunk, then corner-major within chunk:
    j = b*NIDX_B + 4*off(g) + k*ng + ii.
    """
    r = rois_n.astype(np.float32)
    g = np.arange(POOL, dtype=np.float32) / np.float32(POOL - 1.0)
    y1 = r[:, 0] / np.float32(IH - 1.0)
    x1 = r[:, 1] / np.float32(IW - 1.0)
    y2 = r[:, 2] / np.float32(IH - 1.0)
    x2 = r[:, 3] / np.float32(IW - 1.0)
    in_y = (y1[:, None] + (y2 - y1)[:, None] * g[None, :]) * np.float32(H - 1.0)
    in_x = (x1[:, None] + (x2 - x1)[:, None] * g[None, :]) * np.float32(W - 1.0)
    val_y = (in_y >= 0.0) & (in_y <= np.float32(H - 1.0))
    val_x = (in_x >= 0.0) & (in_x <= np.float32(W - 1.0))
    y0f = np.floor(in_y)
    x0f = np.floor(in_x)
    y0 = np.clip(y0f, 0, H - 1).astype(np.int64)
    x0 = np.clip(x0f, 0, W - 1).astype(np.int64)
    yb = np.minimum(y0 + 1, H - 1)
    xb = np.minimum(x0 + 1, W - 1)
    ly = (in_y - y0f).astype(np.float32)
    lx = (in_x - x0f).astype(np.float32)
    ay = (1.0 - ly) * val_y
    by = ly * val_y
    ax = (1.0 - lx) * val_x
    bx = lx * val_x

    iy0 = (y0 * W)[:, :, None]
    iyb = (yb * W)[:, :, None]
    jx0 = x0[:, None, :]
    jxb = xb[:, None, :]
    # [4, R, 49] corner indices / weights in (r, pt) point order
    idx4 = np.stack([iy0 + jx0, iy0 + jxb, iyb + jx0, iyb + jxb]).reshape(
        4, R_CORE, PTS)
    w4 = np.stack([ay[:, :, None] * ax[:, None, :],
                   ay[:, :, None] * bx[:, None, :],
                   by[:, :, None] * ax[:, None, :],
                   by[:, :, None] * bx[:, None, :]]).reshape(4, R_CORE, PTS)

    idx_flat = np.zeros(J_TOT, dtype=np.int16)
    w_flat = np.zeros(J_TOT, dtype=np.float16)
    for b in range(NBATCH):
        # flat per-batch point arrays [4, NP]
        ib = idx4[:, b * RB:(b + 1) * RB].reshape(4, NP)
        wb = w4[:, b * RB:(b + 1) * RB].reshape(4, NP)
        j0 = b * NIDX_B
        off = 0
        for ng in CHUNKS:
            nreal = min(ng, NP - off) if off < NP else 0
            for k in range(4):
                o = j0 + 4 * off + k * ng
                if nreal > 0:
                    idx_flat[o:o + nreal] = ib[k, off:off + nreal]
                    w_flat[o:o + nreal] = wb[k, off:off + nreal]
            off += ng

    # wrapped layout per batch: within batch, idx j at partition j%16,
    # slot j//16 (matches per-chunk gather slices since chunk NIDX % 16 == 0)
    idxw = np.empty((128, S_TOT), dtype=np.int16)
    for b in range(NBATCH):
        blk = idx_flat[b * NIDX_B:(b + 1) * NIDX_B].reshape(SPB, 16).T
        idxw[:, b * SPB:(b + 1) * SPB] = np.tile(blk, (8, 1))
    return idxw, w_flat.reshape(1, J_TOT)


def kernel(img: np.ndarray, rois: np.ndarray,
           input_image: np.ndarray) -> np.ndarray:
    from concourse.bass_utils import run_bass_kernel_spmd

    nc = _get_program()
    ones = np.ones((1, 128), dtype=np.float16)
    in_maps = []
    for c in range(N_CORES):
        n, half = c // 2, c % 2
        imgt = np.ascontiguousarray(
            img[n].reshape(C, HW)[_CHAN_OF_POS, :].T).astype(np.float16)
        idxw, wrow = _host_tables(
            rois[n, half * R_CORE:(half + 1) * R_CORE])
        in_maps.append({
            "imgt": imgt,
            "idxt": idxw,
            "wrow": wrow,
            "ones": ones,
        })
    res = run_bass_kernel_spmd(nc, in_maps, core_ids=list(range(N_CORES)))
    out = np.empty((N, B, C, POOL, POOL), dtype=np.float32)
    for c in range(N_CORES):
        n, half = c // 2, c % 2
        out[n, half * R_CORE:(half + 1) * R_CORE] = \
            res.results[c]["out"].reshape(R_CORE, C, POOL, POOL)
    return out


# revision 9
# speedup vs baseline: 3.3202x; 1.0829x over previous
"""CropAndResize (tf.image.crop_and_resize semantics, bilinear, extrap=0)
Trainium2 Bass kernel, data-parallel over 8 NeuronCores.

Full inputs:  img (4,512,64,64) f32, rois (4,300,4) f32, input_image (4,3,1024,1024) f32
Full output:  (4,300,512,7,7) f32

Sharding: core c handles image n = c//2, roi slice [(c%2)*150 : +150].

Host prep (numpy, per core):
  - imgt[hw, q] fp16 token table: payload position q = e*128+p holds channel
    4p+e (so after the transpose-gather, SBUF partition p carries the four
    adjacent channels 4p..4p+3 -> 784B-contiguous output descriptors).
  - Sample coords/weights mirror the reference math in f32; the validity
    mask and lerp factors fold into one fp16 weight per (corner, point).
  - Gather indices in the dma_gather wrapped-int16 layout, weights as a
    flat f16 row. 10 out-batches of 15 rois (735 points padded to 736);
    each out-batch gathers in 6 chunks (5x128 + 1x96 points, corner-major
    within a chunk) to stay under the 512-descriptor SWDGE ring limit.

Device program (per core, per out-batch):
  1. 6 dma_gathers (transpose mode) straight from DRAM imgt ->
     tk[p, e, (k, i)] fp16 per chunk.
  2. Per chunk: PE ones-matmul broadcasts the weight row (PSUM),
     Activation copies PSUM -> fp16 wk; DVE multiplies the whole chunk by
     its weights in one op, writing into prod[p, e, k, i] (corner-planar).
  3. DVE sums the 4 corner planes (3 adds, fp16 2x).
  4. Activation casts/permutes acc[p, e, (r,pt)] -> ob2[p, r, (e,pt)] f32.
  5. sync DMA ob2 -> out[r0:r0+15] with 784B contiguous descriptors.
"""

import os
import sys

import numpy as np

_RL_REPO_CANDIDATES = ["/opt/trn_rl_repo", "/root/.axon_site/_ro/trn_rl_repo"]
for _p in _RL_REPO_CANDIDATES:
    if os.path.isdir(_p) and _p not in sys.path:
        sys.path.insert(0, _p)

# ---------------------------------------------------------------- constants
N_CORES = 8
N, C, H, W = 4, 512, 64, 64
B = 300
POOL = 7
PTS = POOL * POOL          # 49
IH, IW = 1024.0, 1024.0
R_CORE = B // 2            # 150 rois per core
HW = H * W                 # 4096

RB = 15                    # rois per out-batch
NBATCH = R_CORE // RB      # 10
NP = RB * PTS              # 735 points per out-batch
NP_PAD = 736               # padded (mult of 32)
CHUNKS = (128, 128, 128, 128, 128, 96)   # points per gather chunk
assert sum(CHUNKS) == NP_PAD
NIDX_B = 4 * NP_PAD        # 2944 gather rows per out-batch
SPB = NIDX_B // 16         # 184 wrapped slots per out-batch
S_TOT = NBATCH * SPB       # 1840
J_TOT = NBATCH * NIDX_B    # 29440

_prog_cache = {}


def _build_program():
    import concourse.bass as bass
    import concourse.bacc as bacc
    import concourse.mybir as mybir
    import concourse.tile as tile

    f32 = mybir.dt.float32
    f16 = mybir.dt.float16
    i16 = mybir.dt.int16
    Alu = mybir.AluOpType

    nc = bacc.Bacc("TRN2", target_bir_lowering=False, debug=False,
                   num_devices=N_CORES)

    imgt = nc.dram_tensor("imgt", (HW, C), f16, kind="ExternalInput")
    idxt = nc.dram_tensor("idxt", (128, S_TOT), i16, kind="ExternalInput")
    wrow_d = nc.dram_tensor("wrow", (1, J_TOT), f16, kind="ExternalInput")
    ones_d = nc.dram_tensor("ones", (1, 128), f16, kind="ExternalInput")
    # partition-major fp16 output [p, r, e, q]; host unpermutes to
    # (r, 4p+e, q) and upcasts -- halves output DMA bytes with >=512B descs
    out_t = nc.dram_tensor("out", (128, R_CORE * 4 * PTS), f16,
                           kind="ExternalOutput")

    with tile.TileContext(nc) as tc:
        _body(tc, nc, bass, mybir, tile, imgt, idxt, wrow_d, ones_d, out_t,
              f32, f16, i16, Alu)

    nc.compile()
    return nc


def _body(tc, nc, bass, mybir, tile, imgt, idxt, wrow_d, ones_d, out_t,
          f32, f16, i16, Alu):
    from contextlib import ExitStack
    ctx = ExitStack()
    with ctx:
        const_pool = ctx.enter_context(tc.tile_pool(name="const", bufs=1))
        gather_pool = ctx.enter_context(tc.tile_pool(name="gather", bufs=2))
        wk_pool = ctx.enter_context(tc.tile_pool(name="wk", bufs=2))
        prod_pool = ctx.enter_context(tc.tile_pool(name="prod", bufs=2))
        acc_pool = ctx.enter_context(tc.tile_pool(name="acc", bufs=2))
        ob_pool = ctx.enter_context(tc.tile_pool(name="ob", bufs=2))
        psum_pool = ctx.enter_context(
            tc.tile_pool(name="psum", bufs=1, space="PSUM"))

        # ---- constants; idx split so batch 0 can gather immediately
        idx_s = const_pool.tile([128, S_TOT], i16, tag="idx")
        nc.sync.dma_start(idx_s[:, 0:SPB], idxt.ap()[:, 0:SPB])
        nc.sync.dma_start(idx_s[:, SPB:], idxt.ap()[:, SPB:])
        ones_s = const_pool.tile([1, 128], f16, tag="ones")
        nc.sync.dma_start(ones_s[:, :], ones_d.ap()[:, :])
        # all corner weights resident (one small DMA; keeps SP free of the
        # per-batch load that would queue behind output DMAs)
        wr_all = const_pool.tile([1, J_TOT], f16, tag="wr")
        nc.sync.dma_start(wr_all[:, :], wrow_d.ap()[:, :])

        for b in range(NBATCH):
            r0 = b * RB

            # prod[p, e, k, i]: corner-planar weighted gather products
            prod = prod_pool.tile([128, 4, 4, NP_PAD], f16, tag="P")

            off = 0  # point offset within the out-batch
            for g, ng in enumerate(CHUNKS):
                nidx = 4 * ng
                s0 = b * SPB + off * 4 // 16
                j0 = b * NIDX_B + 4 * off
                tkg = gather_pool.tile([128, 4, nidx], f16, tag=f"T{ng}",
                                       bufs=6 if ng == 128 else 2)
                nc.gpsimd.dma_gather(
                    tkg[:, :, :], imgt.ap()[:, :],
                    idx_s[:, s0:s0 + nidx // 16],
                    nidx, nidx, C, transpose=True,
                )
                ps = psum_pool.tile([128, nidx], f32, tag=f"ps{ng}",
                                    bufs=5 if ng == 128 else 2)
                nc.tensor.matmul(ps[:, :], ones_s[:, :],
                                 wr_all[:, j0:j0 + nidx],
                                 start=True, stop=True)
                wk = wk_pool.tile([128, nidx], f16, tag=f"wk{ng}",
                                  bufs=6 if ng == 128 else 2)
                nc.scalar.copy(wk[:, :], ps[:, :])
                wkb = wk[:, :].unsqueeze(1).broadcast_to([128, 4, nidx])
                # one mul per chunk; dst view splits (k,i) into planes
                src = tkg[:, :, :].rearrange("p e (k i) -> p e k i", k=4)
                dst = prod[:, :, :, off:off + ng]
                nc.vector.tensor_tensor(
                    dst, src, wkb.rearrange("p e (k i) -> p e k i", k=4),
                    Alu.mult)
                off += ng

            acc = acc_pool.tile([128, 4, NP_PAD], f16, tag="A")
            ob16 = ob_pool.tile([128, RB, 4, PTS], f16, tag="O")
            # last batch: split the drain chain in two so the final
            # gather's dependents are short
            if b == NBATCH - 1:
                halves = ((0, 384, 0, 7), (384, NP_PAD, 7, RB))
            else:
                halves = ((0, NP_PAD, 0, RB),)
            for (i0, i1, ra, rb_) in halves:
                # corner reduction: acc = P0+P1+P2; the final add writes
                # the (r, e, q)-permuted fp16 output view directly (2x)
                nc.vector.tensor_tensor(
                    acc[:, :, i0:i1], prod[:, :, 0, i0:i1],
                    prod[:, :, 1, i0:i1], Alu.add)
                nc.vector.tensor_tensor(
                    acc[:, :, i0:i1], acc[:, :, i0:i1],
                    prod[:, :, 2, i0:i1], Alu.add)
                accv = acc[:, :, ra * PTS:rb_ * PTS].rearrange(
                    "p e (r q) -> p e r q", r=rb_ - ra)
                p3v = prod[:, :, 3, ra * PTS:rb_ * PTS].rearrange(
                    "p e (r q) -> p e r q", r=rb_ - ra)
                dstv = ob16[:, ra:rb_, :, :].rearrange(
                    "p r e q -> p e r q")
                nc.vector.tensor_tensor(dstv, accv, p3v, Alu.add)
                # output write: contiguous (r, e, q) streams per partition
                dram = out_t.ap()[:, (r0 + ra) * 4 * PTS:
                                  (r0 + rb_) * 4 * PTS]
                nc.sync.dma_start(
                    dram, ob16[:, ra:rb_, :, :].rearrange(
                        "p r e q -> p (r e q)"))


def _get_program():
    if "nc" not in _prog_cache:
        _prog_cache["nc"] = _build_program()
    return _prog_cache["nc"]


# Channel permutation: payload position q = e*128+p holds channel 4p+e.
_POS = np.arange(C)
_CHAN_OF_POS = 4 * (_POS % 128) + _POS // 128  # [512] channel at position q


def _host_tables(rois_n: np.ndarray):
    """Mirror the reference coordinate math in f32; return wrapped int16
    gather indices [128, S_TOT] and folded fp16 corner weights [1, J_TOT].

    Flat j order: batch-major, then chunk, then corner-major within chunk:
    j = b*NIDX_B + 4*off(g) + k*ng + ii.
    """
    r = rois_n.astype(np.float32)
    g = np.arange(POOL, dtype=np.float32) / np.float32(POOL - 1.0)
    y1 = r[:, 0] / np.float32(IH - 1.0)
    x1 = r[:, 1] / np.float32(IW - 1.0)
    y2 = r[:, 2] / np.float32(IH - 1.0)
    x2 = r[:, 3] / np.float32(IW - 1.0)
    in_y = (y1[:, None] + (y2 - y1)[:, None] * g[None, :]) * np.float32(H - 1.0)
    in_x = (x1[:, None] + (x2 - x1)[:, None] * g[None, :]) * np.float32(W - 1.0)
    val_y = (in_y >= 0.0) & (in_y <= np.float32(H - 1.0))
    val_x = (in_x >= 0.0) & (in_x <= np.float32(W - 1.0))
    y0f = np.floor(in_y)
    x0f = np.floor(in_x)
    y0 = np.clip(y0f, 0, H - 1).astype(np.int64)
    x0 = np.clip(x0f, 0, W - 1).astype(np.int64)
    yb = np.minimum(y0 + 1, H - 1)
    xb = np.minimum(x0 + 1, W - 1)
    ly = (in_y - y0f).astype(np.float32)
    lx = (in_x - x0f).astype(np.float32)
    ay = (1.0 - ly) * val_y
    by = ly * val_y
    ax = (1.0 - lx) * val_x
    bx = lx * val_x

    iy0 = (y0 * W)[:, :, None]
    iyb = (yb * W)[:, :, None]
    jx0 = x0[:, None, :]
    jxb = xb[:, None, :]
    # [4, R, 49] corner indices / weights in (r, pt) point order
    idx4 = np.stack([iy0 + jx0, iy0 + jxb, iyb + jx0, iyb + jxb]).reshape(
        4, R_CORE, PTS)
    w4 = np.stack([ay[:, :, None] * ax[:, None, :],
                   ay[:, :, None] * bx[:, None, :],
                   by[:, :, None] * ax[:, None, :],
                   by[:, :, None] * bx[:, None, :]]).reshape(4, R_CORE, PTS)

    idx_flat = np.zeros(J_TOT, dtype=np.int16)
    w_flat = np.zeros(J_TOT, dtype=np.float16)
    for b in range(NBATCH):
        # flat per-batch point arrays [4, NP]
        ib = idx4[:, b * RB:(b + 1) * RB].reshape(4, NP)
        wb = w4[:, b * RB:(b + 1) * RB].reshape(4, NP)
        j0 = b * NIDX_B
        off = 0
        for ng in CHUNKS:
            nreal = min(ng, NP - off) if off < NP else 0
            for k in range(4):
                o = j0 + 4 * off + k * ng
                if nreal > 0:
                    idx_flat[o:o + nreal] = ib[k, off:off + nreal]
                    w_flat[o:o + nreal] = wb[k, off:off + nreal]
            off += ng

    # wrapped layout per batch: within batch, idx j at partition j%16,
    # slot j//16 (matches per-chunk gather slices since chunk NIDX % 16 == 0)
    idxw = np.empty((128, S_TOT), dtype=np.int16)
    for b in range(NBATCH):
        blk = idx_flat[b * NIDX_B:(b + 1) * NIDX_B].reshape(SPB, 16).T
        idxw[:, b * SPB:(b + 1) * SPB] = np.tile(blk, (8, 1))
    return idxw, w_flat.reshape(1, J_TOT)


def kernel(img: np.ndarray, rois: np.ndarray,
           input_image: np.ndarray) -> np.ndarray:
    from concourse.bass_utils import run_bass_kernel_spmd

    nc = _get_program()
    ones = np.ones((1, 128), dtype=np.float16)
    in_maps = []
    for c in range(N_CORES):
        n, half = c // 2, c % 2
        imgt = np.ascontiguousarray(
            img[n].reshape(C, HW)[_CHAN_OF_POS, :].T).astype(np.float16)
        idxw, wrow = _host_tables(
            rois[n, half * R_CORE:(half + 1) * R_CORE])
        in_maps.append({
            "imgt": imgt,
            "idxt": idxw,
            "wrow": wrow,
            "ones": ones,
        })
    res = run_bass_kernel_spmd(nc, in_maps, core_ids=list(range(N_CORES)))
    out = np.empty((N, B, C, POOL, POOL), dtype=np.float32)
    for c in range(N_CORES):
        n, half = c // 2, c % 2
        # device buffer is [p, r, e, q] fp16 with channel c = 4p+e
        buf = res.results[c]["out"].reshape(128, R_CORE, 4, PTS)
        out[n, half * R_CORE:(half + 1) * R_CORE] = (
            buf.transpose(1, 0, 2, 3).reshape(R_CORE, C, POOL, POOL)
            .astype(np.float32))
    return out


# revision 12
# speedup vs baseline: 3.3234x; 1.0010x over previous
"""CropAndResize (tf.image.crop_and_resize semantics, bilinear, extrap=0)
Trainium2 Bass kernel, data-parallel over 8 NeuronCores.

Full inputs:  img (4,512,64,64) f32, rois (4,300,4) f32, input_image (4,3,1024,1024) f32
Full output:  (4,300,512,7,7) f32

Sharding: core c handles image n = c//2, roi slice [(c%2)*150 : +150].

Host prep (numpy, per core):
  - imgt[hw, q] fp16 token table: payload position q = e*128+p holds channel
    4p+e (so after the transpose-gather, SBUF partition p carries the four
    adjacent channels 4p..4p+3 -> 784B-contiguous output descriptors).
  - Sample coords/weights mirror the reference math in f32; the validity
    mask and lerp factors fold into one fp16 weight per (corner, point).
  - Gather indices in the dma_gather wrapped-int16 layout, weights as a
    flat f16 row. 10 out-batches of 15 rois (735 points padded to 736);
    each out-batch gathers in 6 chunks (5x128 + 1x96 points, corner-major
    within a chunk) to stay under the 512-descriptor SWDGE ring limit.

Device program (per core, per out-batch):
  1. 6 dma_gathers (transpose mode) straight from DRAM imgt ->
     tk[p, e, (k, i)] fp16 per chunk.
  2. Per chunk: PE ones-matmul broadcasts the weight row (PSUM),
     Activation copies PSUM -> fp16 wk; DVE multiplies the whole chunk by
     its weights in one op, writing into prod[p, e, k, i] (corner-planar).
  3. DVE sums the 4 corner planes (3 adds, fp16 2x).
  4. Activation casts/permutes acc[p, e, (r,pt)] -> ob2[p, r, (e,pt)] f32.
  5. sync DMA ob2 -> out[r0:r0+15] with 784B contiguous descriptors.
"""

import os
import sys

import numpy as np

_RL_REPO_CANDIDATES = ["/opt/trn_rl_repo", "/root/.axon_site/_ro/trn_rl_repo"]
for _p in _RL_REPO_CANDIDATES:
    if os.path.isdir(_p) and _p not in sys.path:
        sys.path.insert(0, _p)

# ---------------------------------------------------------------- constants
N_CORES = 8
N, C, H, W = 4, 512, 64, 64
B = 300
POOL = 7
PTS = POOL * POOL          # 49
IH, IW = 1024.0, 1024.0
R_CORE = B // 2            # 150 rois per core
HW = H * W                 # 4096

RB = 15                    # rois per out-batch
NBATCH = R_CORE // RB      # 10
NP = RB * PTS              # 735 points per out-batch
NP_PAD = 736               # padded (mult of 32)
CHUNKS = (64, 224, 224, 224)             # points per gather chunk
assert sum(CHUNKS) == NP_PAD
NIDX_B = 4 * NP_PAD        # 2944 gather rows per out-batch
SPB = NIDX_B // 16         # 184 wrapped slots per out-batch
S_TOT = NBATCH * SPB       # 1840
J_TOT = NBATCH * NIDX_B    # 29440

_prog_cache = {}


def _build_program():
    import concourse.bass as bass
    import concourse.bacc as bacc
    import concourse.mybir as mybir
    import concourse.tile as tile

    f32 = mybir.dt.float32
    f16 = mybir.dt.float16
    i16 = mybir.dt.int16
    Alu = mybir.AluOpType

    nc = bacc.Bacc("TRN2", target_bir_lowering=False, debug=False,
                   num_devices=N_CORES)

    imgt = nc.dram_tensor("imgt", (HW, C), f16, kind="ExternalInput")
    idxt = nc.dram_tensor("idxt", (128, S_TOT), i16, kind="ExternalInput")
    wrow_d = nc.dram_tensor("wrow", (1, J_TOT), f16, kind="ExternalInput")
    ones_d = nc.dram_tensor("ones", (1, 128), f16, kind="ExternalInput")
    # partition-major fp16 output [p, r, e, q]; host unpermutes to
    # (r, 4p+e, q) and upcasts -- halves output DMA bytes with >=512B descs
    out_t = nc.dram_tensor("out", (128, R_CORE * 4 * PTS), f16,
                           kind="ExternalOutput")

    with tile.TileContext(nc) as tc:
        _body(tc, nc, bass, mybir, tile, imgt, idxt, wrow_d, ones_d, out_t,
              f32, f16, i16, Alu)

    nc.compile()
    return nc


def _body(tc, nc, bass, mybir, tile, imgt, idxt, wrow_d, ones_d, out_t,
          f32, f16, i16, Alu):
    from contextlib import ExitStack
    ctx = ExitStack()
    with ctx:
        const_pool = ctx.enter_context(tc.tile_pool(name="const", bufs=1))
        gather_pool = ctx.enter_context(tc.tile_pool(name="gather", bufs=2))
        wk_pool = ctx.enter_context(tc.tile_pool(name="wk", bufs=2))
        prod_pool = ctx.enter_context(tc.tile_pool(name="prod", bufs=2))
        acc_pool = ctx.enter_context(tc.tile_pool(name="acc", bufs=2))
        ob_pool = ctx.enter_context(tc.tile_pool(name="ob", bufs=2))
        psum_pool = ctx.enter_context(
            tc.tile_pool(name="psum", bufs=1, space="PSUM"))

        # ---- constants; idx split so batch 0 can gather immediately
        idx_s = const_pool.tile([128, S_TOT], i16, tag="idx")
        nc.sync.dma_start(idx_s[:, 0:SPB], idxt.ap()[:, 0:SPB])
        nc.sync.dma_start(idx_s[:, SPB:], idxt.ap()[:, SPB:])
        ones_s = const_pool.tile([1, 128], f16, tag="ones")
        nc.sync.dma_start(ones_s[:, :], ones_d.ap()[:, :])
        # all corner weights resident (one small DMA; keeps SP free of the
        # per-batch load that would queue behind output DMAs)
        wr_all = const_pool.tile([1, J_TOT], f16, tag="wr")
        nc.sync.dma_start(wr_all[:, :], wrow_d.ap()[:, :])

        for b in range(NBATCH):
            r0 = b * RB

            # prod[p, e, k, i]: corner-planar weighted gather products
            prod = prod_pool.tile([128, 4, 4, NP_PAD], f16, tag="P")

            off = 0  # point offset within the out-batch
            for g, ng in enumerate(CHUNKS):
                nidx = 4 * ng
                s0 = b * SPB + off * 4 // 16
                j0 = b * NIDX_B + 4 * off
                tkg = gather_pool.tile([128, 4, nidx], f16, tag=f"T{ng}",
                                       bufs=4 if ng == 224 else 2)
                nc.gpsimd.dma_gather(
                    tkg[:, :, :], imgt.ap()[:, :],
                    idx_s[:, s0:s0 + nidx // 16],
                    nidx, nidx, C, transpose=True,
                )
                wk = wk_pool.tile([128, nidx], f16, tag=f"wk{ng}",
                                  bufs=4 if ng == 224 else 2)
                nh = (nidx + 511) // 512  # matmul N capped by one PSUM bank
                hp = nidx // nh
                for h in range(nh):
                    ps = psum_pool.tile([128, hp], f32, tag=f"ps{hp}",
                                        bufs=4 if ng == 224 else 2)
                    nc.tensor.matmul(
                        ps[:, :], ones_s[:, :],
                        wr_all[:, j0 + h * hp:j0 + (h + 1) * hp],
                        start=True, stop=True)
                    nc.scalar.copy(wk[:, h * hp:(h + 1) * hp], ps[:, :])
                wkb = wk[:, :].unsqueeze(1).broadcast_to([128, 4, nidx])
                # one mul per chunk; dst view splits (k,i) into planes
                src = tkg[:, :, :].rearrange("p e (k i) -> p e k i", k=4)
                dst = prod[:, :, :, off:off + ng]
                nc.vector.tensor_tensor(
                    dst, src, wkb.rearrange("p e (k i) -> p e k i", k=4),
                    Alu.mult)
                off += ng

            acc = acc_pool.tile([128, 4, NP_PAD], f16, tag="A")
            ob16 = ob_pool.tile([128, RB, 4, PTS], f16, tag="O")
            # last batch: split the drain chain in two so the final
            # gather's dependents are short
            if b == NBATCH - 1:
                halves = ((0, 512, 0, 7), (512, NP_PAD, 7, RB))
            else:
                halves = ((0, NP_PAD, 0, RB),)
            for (i0, i1, ra, rb_) in halves:
                # corner reduction: acc = P0+P1+P2; the final add writes
                # the (r, e, q)-permuted fp16 output view directly (2x)
                nc.vector.tensor_tensor(
                    acc[:, :, i0:i1], prod[:, :, 0, i0:i1],
                    prod[:, :, 1, i0:i1], Alu.add)
                nc.vector.tensor_tensor(
                    acc[:, :, i0:i1], acc[:, :, i0:i1],
                    prod[:, :, 2, i0:i1], Alu.add)
                accv = acc[:, :, ra * PTS:rb_ * PTS].rearrange(
                    "p e (r q) -> p e r q", r=rb_ - ra)
                p3v = prod[:, :, 3, ra * PTS:rb_ * PTS].rearrange(
                    "p e (r q) -> p e r q", r=rb_ - ra)
                dstv = ob16[:, ra:rb_, :, :].rearrange(
                    "p r e q -> p e r q")
                nc.vector.tensor_tensor(dstv, accv, p3v, Alu.add)
                # output write: contiguous (r, e, q) streams per partition
                dram = out_t.ap()[:, (r0 + ra) * 4 * PTS:
                                  (r0 + rb_) * 4 * PTS]
                nc.sync.dma_start(
                    dram, ob16[:, ra:rb_, :, :].rearrange(
                        "p r e q -> p (r e q)"))


def _get_program():
    if "nc" not in _prog_cache:
        _prog_cache["nc"] = _build_program()
    return _prog_cache["nc"]


# Channel permutation: payload position q = e*128+p holds channel 4p+e.
_POS = np.arange(C)
_CHAN_OF_POS = 4 * (_POS % 128) + _POS // 128  # [512] channel at position q


def _host_tables(rois_n: np.ndarray):
    """Mirror the reference coordinate math in f32; return wrapped int16
    gather indices [128, S_TOT] and folded fp16 corner weights [1, J_TOT].

    Flat j order: batch-major, then chunk, then corner-major within chunk:
    j = b*NIDX_B + 4*off(g) + k*ng + ii.
    """
    r = rois_n.astype(np.float32)
    g = np.arange(POOL, dtype=np.float32) / np.float32(POOL - 1.0)
    y1 = r[:, 0] / np.float32(IH - 1.0)
    x1 = r[:, 1] / np.float32(IW - 1.0)
    y2 = r[:, 2] / np.float32(IH - 1.0)
    x2 = r[:, 3] / np.float32(IW - 1.0)
    in_y = (y1[:, None] + (y2 - y1)[:, None] * g[None, :]) * np.float32(H - 1.0)
    in_x = (x1[:, None] + (x2 - x1)[:, None] * g[None, :]) * np.float32(W - 1.0)
    val_y = (in_y >= 0.0) & (in_y <= np.float32(H - 1.0))
    val_x = (in_x >= 0.0) & (in_x <= np.float32(W - 1.0))
    y0f = np.floor(in_y)
    x0f = np.floor(in_x)
    y0 = np.clip(y0f, 0, H - 1).astype(np.int64)
    x0 = np.clip(x0f, 0, W - 1).astype(np.int64)
    yb = np.minimum(y0 + 1, H - 1)
    xb = np.minimum(x0 + 1, W - 1)
    ly = (in_y - y0f).astype(np.float32)
    lx = (in_x - x0f).astype(np.float32)
    ay = (1.0 - ly) * val_y
    by = ly * val_y
    ax = (1.0 - lx) * val_x
    bx = lx * val_x

    iy0 = (y0 * W)[:, :, None]
    iyb = (yb * W)[:, :, None]
    jx0 = x0[:, None, :]
    jxb = xb[:, None, :]
    # [4, R, 49] corner indices / weights in (r, pt) point order
    idx4 = np.stack([iy0 + jx0, iy0 + jxb, iyb + jx0, iyb + jxb]).reshape(
        4, R_CORE, PTS)
    w4 = np.stack([ay[:, :, None] * ax[:, None, :],
                   ay[:, :, None] * bx[:, None, :],
                   by[:, :, None] * ax[:, None, :],
                   by[:, :, None] * bx[:, None, :]]).reshape(4, R_CORE, PTS)

    idx_flat = np.zeros(J_TOT, dtype=np.int16)
    w_flat = np.zeros(J_TOT, dtype=np.float16)
    for b in range(NBATCH):
        # flat per-batch point arrays [4, NP]
        ib = idx4[:, b * RB:(b + 1) * RB].reshape(4, NP)
        wb = w4[:, b * RB:(b + 1) * RB].reshape(4, NP)
        j0 = b * NIDX_B
        off = 0
        for ng in CHUNKS:
            nreal = min(ng, NP - off) if off < NP else 0
            for k in range(4):
                o = j0 + 4 * off + k * ng
                if nreal > 0:
                    idx_flat[o:o + nreal] = ib[k, off:off + nreal]
                    w_flat[o:o + nreal] = wb[k, off:off + nreal]
            off += ng

    # wrapped layout per batch: within batch, idx j at partition j%16,
    # slot j//16 (matches per-chunk gather slices since chunk NIDX % 16 == 0)
    idxw = np.empty((128, S_TOT), dtype=np.int16)
    for b in range(NBATCH):
        blk = idx_flat[b * NIDX_B:(b + 1) * NIDX_B].reshape(SPB, 16).T
        idxw[:, b * SPB:(b + 1) * SPB] = np.tile(blk, (8, 1))
    return idxw, w_flat.reshape(1, J_TOT)


def kernel(img: np.ndarray, rois: np.ndarray,
           input_image: np.ndarray) -> np.ndarray:
    from concourse.bass_utils import run_bass_kernel_spmd

    nc = _get_program()
    ones = np.ones((1, 128), dtype=np.float16)
    in_maps = []
    for c in range(N_CORES):
        n, half = c // 2, c % 2
        imgt = np.ascontiguousarray(
            img[n].reshape(C, HW)[_CHAN_OF_POS, :].T).astype(np.float16)
        idxw, wrow = _host_tables(
            rois[n, half * R_CORE:(half + 1) * R_CORE])
        in_maps.append({
            "imgt": imgt,
            "idxt": idxw,
            "wrow": wrow,
            "ones": ones,
        })
    res = run_bass_kernel_spmd(nc, in_maps, core_ids=list(range(N_CORES)))
    out = np.empty((N, B, C, POOL, POOL), dtype=np.float32)
    for c in range(N_CORES):
        n, half = c // 2, c % 2
        # device buffer is [p, r, e, q] fp16 with channel c = 4p+e
        buf = res.results[c]["out"].reshape(128, R_CORE, 4, PTS)
        out[n, half * R_CORE:(half + 1) * R_CORE] = (
            buf.transpose(1, 0, 2, 3).reshape(R_CORE, C, POOL, POOL)
            .astype(np.float32))
    return out


# revision 13
# speedup vs baseline: 3.4317x; 1.0326x over previous
"""CropAndResize (tf.image.crop_and_resize semantics, bilinear, extrap=0)
Trainium2 Bass kernel, data-parallel over 8 NeuronCores.

Full inputs:  img (4,512,64,64) f32, rois (4,300,4) f32, input_image (4,3,1024,1024) f32
Full output:  (4,300,512,7,7) f32

Sharding: core c handles image n = c//2, roi slice [(c%2)*150 : +150].

Host prep (numpy, per core):
  - imgt[hw, q] fp16 token table: payload position q = e*128+p holds channel
    4p+e (so after the transpose-gather, SBUF partition p carries the four
    adjacent channels 4p..4p+3 -> 784B-contiguous output descriptors).
  - Sample coords/weights mirror the reference math in f32; the validity
    mask and lerp factors fold into one fp16 weight per (corner, point).
  - Gather indices in the dma_gather wrapped-int16 layout, weights as a
    flat f16 row. 10 out-batches of 15 rois (735 points padded to 736);
    each out-batch gathers in 6 chunks (5x128 + 1x96 points, corner-major
    within a chunk) to stay under the 512-descriptor SWDGE ring limit.

Device program (per core, per out-batch):
  1. 6 dma_gathers (transpose mode) straight from DRAM imgt ->
     tk[p, e, (k, i)] fp16 per chunk.
  2. Per chunk: PE ones-matmul broadcasts the weight row (PSUM),
     Activation copies PSUM -> fp16 wk; DVE multiplies the whole chunk by
     its weights in one op, writing into prod[p, e, k, i] (corner-planar).
  3. DVE sums the 4 corner planes (3 adds, fp16 2x).
  4. Activation casts/permutes acc[p, e, (r,pt)] -> ob2[p, r, (e,pt)] f32.
  5. sync DMA ob2 -> out[r0:r0+15] with 784B contiguous descriptors.
"""

import os
import sys

import numpy as np

_RL_REPO_CANDIDATES = ["/opt/trn_rl_repo", "/root/.axon_site/_ro/trn_rl_repo"]
for _p in _RL_REPO_CANDIDATES:
    if os.path.isdir(_p) and _p not in sys.path:
        sys.path.insert(0, _p)

# ---------------------------------------------------------------- constants
N_CORES = 8
N, C, H, W = 4, 512, 64, 64
B = 300
POOL = 7
PTS = POOL * POOL          # 49
IH, IW = 1024.0, 1024.0
R_CORE = B // 2            # 150 rois per core
HW = H * W                 # 4096

RB = 15                    # rois per out-batch
NBATCH = R_CORE // RB      # 10
NP = RB * PTS              # 735 points per out-batch
NP_PAD = 736               # padded (mult of 32)
CHUNKS = (64, 224, 224, 224)             # points per gather chunk
assert sum(CHUNKS) == NP_PAD
NIDX_B = 4 * NP_PAD        # 2944 gather rows per out-batch
SPB = NIDX_B // 16         # 184 wrapped slots per out-batch
S_TOT = NBATCH * SPB       # 1840
J_TOT = NBATCH * NIDX_B    # 29440

_prog_cache = {}


def _build_program():
    import concourse.bass as bass
    import concourse.bacc as bacc
    import concourse.mybir as mybir
    import concourse.tile as tile

    f32 = mybir.dt.float32
    f16 = mybir.dt.float16
    i16 = mybir.dt.int16
    Alu = mybir.AluOpType

    nc = bacc.Bacc("TRN2", target_bir_lowering=False, debug=False,
                   num_devices=N_CORES)

    imgt = nc.dram_tensor("imgt", (HW, C), f16, kind="ExternalInput")
    idxt = nc.dram_tensor("idxt", (128, S_TOT), i16, kind="ExternalInput")
    wrow_d = nc.dram_tensor("wrow", (1, J_TOT), f16, kind="ExternalInput")
    ones_d = nc.dram_tensor("ones", (1, 128), f16, kind="ExternalInput")
    # partition-major fp16 output [p, r, e, q]; host unpermutes to
    # (r, 4p+e, q) and upcasts -- halves output DMA bytes with >=512B descs
    out_t = nc.dram_tensor("out", (128, R_CORE * 4 * PTS), f16,
                           kind="ExternalOutput")

    with tile.TileContext(nc) as tc:
        _body(tc, nc, bass, mybir, tile, imgt, idxt, wrow_d, ones_d, out_t,
              f32, f16, i16, Alu)

    nc.compile()
    return nc


def _body(tc, nc, bass, mybir, tile, imgt, idxt, wrow_d, ones_d, out_t,
          f32, f16, i16, Alu):
    from contextlib import ExitStack
    ctx = ExitStack()
    with ctx:
        const_pool = ctx.enter_context(tc.tile_pool(name="const", bufs=1))
        gather_pool = ctx.enter_context(tc.tile_pool(name="gather", bufs=2))
        wk_pool = ctx.enter_context(tc.tile_pool(name="wk", bufs=2))
        prod_pool = ctx.enter_context(tc.tile_pool(name="prod", bufs=2))
        acc_pool = ctx.enter_context(tc.tile_pool(name="acc", bufs=2))
        ob_pool = ctx.enter_context(tc.tile_pool(name="ob", bufs=2))
        psum_pool = ctx.enter_context(
            tc.tile_pool(name="psum", bufs=1, space="PSUM"))

        # ---- constants; idx split so batch 0 can gather immediately
        idx_s = const_pool.tile([128, S_TOT], i16, tag="idx")
        nc.sync.dma_start(idx_s[:, 0:SPB], idxt.ap()[:, 0:SPB])
        nc.sync.dma_start(idx_s[:, SPB:], idxt.ap()[:, SPB:])
        ones_s = const_pool.tile([1, 128], f16, tag="ones")
        nc.sync.dma_start(ones_s[:, :], ones_d.ap()[:, :])
        # all corner weights resident (one small DMA; keeps SP free of the
        # per-batch load that would queue behind output DMAs)
        wr_all = const_pool.tile([1, J_TOT], f16, tag="wr")
        nc.sync.dma_start(wr_all[:, :], wrow_d.ap()[:, :])

        for b in range(NBATCH):
            r0 = b * RB

            # prod[p, e, k, i]: corner-planar weighted gather products
            prod = prod_pool.tile([128, 4, 4, NP_PAD], f16, tag="P")

            off = 0  # point offset within the out-batch
            for g, ng in enumerate(CHUNKS):
                nidx = 4 * ng
                s0 = b * SPB + off * 4 // 16
                j0 = b * NIDX_B + 4 * off
                tkg = gather_pool.tile([128, 4, nidx], f16, tag=f"T{ng}",
                                       bufs=4 if ng == 224 else 2)
                nc.gpsimd.dma_gather(
                    tkg[:, :, :], imgt.ap()[:, :],
                    idx_s[:, s0:s0 + nidx // 16],
                    nidx, nidx, C, transpose=True,
                )
                wk = wk_pool.tile([128, nidx], f16, tag=f"wk{ng}",
                                  bufs=4 if ng == 224 else 2)
                nh = (nidx + 511) // 512  # matmul N capped by one PSUM bank
                hp = nidx // nh
                for h in range(nh):
                    ps = psum_pool.tile([128, hp], f32, tag=f"ps{hp}",
                                        bufs=4 if ng == 224 else 2)
                    nc.tensor.matmul(
                        ps[:, :], ones_s[:, :],
                        wr_all[:, j0 + h * hp:j0 + (h + 1) * hp],
                        start=True, stop=True)
                    nc.scalar.copy(wk[:, h * hp:(h + 1) * hp], ps[:, :])
                wkb = wk[:, :].unsqueeze(1).broadcast_to([128, 4, nidx])
                # one mul per chunk; dst view splits (k,i) into planes.
                # The small chunk's mul runs on GPSIMD to shorten the DVE
                # critical path (DVE is the bottleneck engine).
                eng = nc.gpsimd if ng == 64 else nc.vector
                src = tkg[:, :, :].rearrange("p e (k i) -> p e k i", k=4)
                dst = prod[:, :, :, off:off + ng]
                eng.tensor_tensor(
                    dst, src, wkb.rearrange("p e (k i) -> p e k i", k=4),
                    Alu.mult)
                off += ng

            acc = acc_pool.tile([128, 4, NP_PAD], f16, tag="A")
            ob16 = ob_pool.tile([128, RB, 4, PTS], f16, tag="O")
            # last batch: split the drain chain in two so the final
            # gather's dependents are short
            if b == NBATCH - 1:
                halves = ((0, 512, 0, 7), (512, NP_PAD, 7, RB))
            else:
                halves = ((0, NP_PAD, 0, RB),)
            for (i0, i1, ra, rb_) in halves:
                # corner reduction: acc = P0+P1+P2; the final add writes
                # the (r, e, q)-permuted fp16 output view directly (2x)
                nc.vector.tensor_tensor(
                    acc[:, :, i0:i1], prod[:, :, 0, i0:i1],
                    prod[:, :, 1, i0:i1], Alu.add)
                nc.vector.tensor_tensor(
                    acc[:, :, i0:i1], acc[:, :, i0:i1],
                    prod[:, :, 2, i0:i1], Alu.add)
                accv = acc[:, :, ra * PTS:rb_ * PTS].rearrange(
                    "p e (r q) -> p e r q", r=rb_ - ra)
                p3v = prod[:, :, 3, ra * PTS:rb_ * PTS].rearrange(
                    "p e (r q) -> p e r q", r=rb_ - ra)
                dstv = ob16[:, ra:rb_, :, :].rearrange(
                    "p r e q -> p e r q")
                nc.vector.tensor_tensor(dstv, accv, p3v, Alu.add)
                # output write: contiguous (r, e, q) streams per partition
                dram = out_t.ap()[:, (r0 + ra) * 4 * PTS:
                                  (r0 + rb_) * 4 * PTS]
                nc.sync.dma_start(
                    dram, ob16[:, ra:rb_, :, :].rearrange(
                        "p r e q -> p (r e q)"))


def _get_program():
    if "nc" not in _prog_cache:
        _prog_cache["nc"] = _build_program()
    return _prog_cache["nc"]


# Channel permutation: payload position q = e*128+p holds channel 4p+e.
_POS = np.arange(C)
_CHAN_OF_POS = 4 * (_POS % 128) + _POS // 128  # [512] channel at position q


def _host_tables(rois_n: np.ndarray):
    """Mirror the reference coordinate math in f32; return wrapped int16
    gather indices [128, S_TOT] and folded fp16 corner weights [1, J_TOT].

    Flat j order: batch-major, then chunk, then corner-major within chunk:
    j = b*NIDX_B + 4*off(g) + k*ng + ii.
    """
    r = rois_n.astype(np.float32)
    g = np.arange(POOL, dtype=np.float32) / np.float32(POOL - 1.0)
    y1 = r[:, 0] / np.float32(IH - 1.0)
    x1 = r[:, 1] / np.float32(IW - 1.0)
    y2 = r[:, 2] / np.float32(IH - 1.0)
    x2 = r[:, 3] / np.float32(IW - 1.0)
    in_y = (y1[:, None] + (y2 - y1)[:, None] * g[None, :]) * np.float32(H - 1.0)
    in_x = (x1[:, None] + (x2 - x1)[:, None] * g[None, :]) * np.float32(W - 1.0)
    val_y = (in_y >= 0.0) & (in_y <= np.float32(H - 1.0))
    val_x = (in_x >= 0.0) & (in_x <= np.float32(W - 1.0))
    y0f = np.floor(in_y)
    x0f = np.floor(in_x)
    y0 = np.clip(y0f, 0, H - 1).astype(np.int64)
    x0 = np.clip(x0f, 0, W - 1).astype(np.int64)
    yb = np.minimum(y0 + 1, H - 1)
    xb = np.minimum(x0 + 1, W - 1)
    ly = (in_y - y0f).astype(np.float32)
    lx = (in_x - x0f).astype(np.float32)
    ay = (1.0 - ly) * val_y
    by = ly * val_y
    ax = (1.0 - lx) * val_x
    bx = lx * val_x

    iy0 = (y0 * W)[:, :, None]
    iyb = (yb * W)[:, :, None]
    jx0 = x0[:, None, :]
    jxb = xb[:, None, :]
    # [4, R, 49] corner indices / weights in (r, pt) point order
    idx4 = np.stack([iy0 + jx0, iy0 + jxb, iyb + jx0, iyb + jxb]).reshape(
        4, R_CORE, PTS)
    w4 = np.stack([ay[:, :, None] * ax[:, None, :],
                   ay[:, :, None] * bx[:, None, :],
                   by[:, :, None] * ax[:, None, :],
                   by[:, :, None] * bx[:, None, :]]).reshape(4, R_CORE, PTS)

    idx_flat = np.zeros(J_TOT, dtype=np.int16)
    w_flat = np.zeros(J_TOT, dtype=np.float16)
    for b in range(NBATCH):
        # flat per-batch point arrays [4, NP]
        ib = idx4[:, b * RB:(b + 1) * RB].reshape(4, NP)
        wb = w4[:, b * RB:(b + 1) * RB].reshape(4, NP)
        j0 = b * NIDX_B
        off = 0
        for ng in CHUNKS:
            nreal = min(ng, NP - off) if off < NP else 0
            for k in range(4):
                o = j0 + 4 * off + k * ng
                if nreal > 0:
                    idx_flat[o:o + nreal] = ib[k, off:off + nreal]
                    w_flat[o:o + nreal] = wb[k, off:off + nreal]
            off += ng

    # wrapped layout per batch: within batch, idx j at partition j%16,
    # slot j//16 (matches per-chunk gather slices since chunk NIDX % 16 == 0)
    idxw = np.empty((128, S_TOT), dtype=np.int16)
    for b in range(NBATCH):
        blk = idx_flat[b * NIDX_B:(b + 1) * NIDX_B].reshape(SPB, 16).T
        idxw[:, b * SPB:(b + 1) * SPB] = np.tile(blk, (8, 1))
    return idxw, w_flat.reshape(1, J_TOT)


def kernel(img: np.ndarray, rois: np.ndarray,
           input_image: np.ndarray) -> np.ndarray:
    from concourse.bass_utils import run_bass_kernel_spmd

    nc = _get_program()
    ones = np.ones((1, 128), dtype=np.float16)
    in_maps = []
    for c in range(N_CORES):
        n, half = c // 2, c % 2
        imgt = np.ascontiguousarray(
            img[n].reshape(C, HW)[_CHAN_OF_POS, :].T).astype(np.float16)
        idxw, wrow = _host_tables(
            rois[n, half * R_CORE:(half + 1) * R_CORE])
        in_maps.append({
            "imgt": imgt,
            "idxt": idxw,
            "wrow": wrow,
            "ones": ones,
        })
    res = run_bass_kernel_spmd(nc, in_maps, core_ids=list(range(N_CORES)))
    out = np.empty((N, B, C, POOL, POOL), dtype=np.float32)
    for c in range(N_CORES):
        n, half = c // 2, c % 2
        # device buffer is [p, r, e, q] fp16 with channel c = 4p+e
        buf = res.results[c]["out"].reshape(128, R_CORE, 4, PTS)
        out[n, half * R_CORE:(half + 1) * R_CORE] = (
            buf.transpose(1, 0, 2, 3).reshape(R_CORE, C, POOL, POOL)
            .astype(np.float32))
    return out
